# revision 2
# baseline (speedup 1.0000x reference)
"""BWGNN v2 on 8 Trainium2 cores.

Structure per core (nodes sharded 8 ways, 12544 padded positions,
98 buckets x 128):
  phase A: h = relu(relu(X W1+b1) W2+b2) computed feature-major
    (weights-as-lhsT), transposed once to node-major; table rows
    tw = h*dinv written to DRAM (unduplicated bf16, row pairs = 256B
    gather elements).
  2 x AllGather per hop (window = half the pair range), chunk-aligned
    with the 49-bucket super-phases so collectives overlap compute.
  hop: dma_gather chunks of 32 blocks (128 pair-elements each; blocks
    are (superphase, window, parity, bucket)-pure), one-hot built per
    block with a single tensor_scalar is_equal (iota vs dst column),
    one matmul per block accumulating agg[128, 49*64] in PSUM.
  epilogue per super-phase: batched dinv scaling + subtract, table
    write for next hop; final dense layers feature-major.
"""
import os
import sys
import types

import numpy as np
import ml_dtypes

import concourse.bacc as bacc
import concourse.bass as bass
import concourse.mybir as mybir
import concourse.tile as tile
from concourse.bass_utils import run_bass_kernel_spmd


def _install_axon_shim():
    if "antenv.axon_hooks" in sys.modules:
        return
    state = {"hook": None}
    mod = types.ModuleType("antenv.axon_hooks")
    mod.set_axon_ntff_profile_hook = lambda h: state.__setitem__("hook", h)
    mod.get_axon_ntff_profile_hook = lambda: state["hook"]
    sys.modules["antenv.axon_hooks"] = mod
    try:
        import antenv

        antenv.axon_hooks = mod
    except Exception:
        pass
    try:
        from trn_agent_boot.trn_boot import _ntff_profile_via_ctypes

        h = _ntff_profile_via_ctypes("/opt/axon/libaxon_pjrt.so")
        if h is not None:
            mod.set_axon_ntff_profile_hook(h)
    except Exception:
        pass


_install_axon_shim()


def split_waits(nc, max_waits=1):
    for f in nc.m.functions:
        for blk in f.blocks:
            new_insts = []
            for inst in blk.instructions:
                si = inst.sync_info
                if si is not None and len(si.on_wait) > max_waits:
                    waits = list(si.on_wait)
                    extra, keep = waits[:-max_waits], waits[-max_waits:]
                    for i in range(0, len(extra), max_waits):
                        nop = mybir.InstNoOp(
                            name=nc.get_next_instruction_name(), ins=[], outs=[]
                        )
                        nop.engine = inst.engine
                        nop.sync_info = mybir.SyncInfo(
                            on_wait=extra[i : i + max_waits], on_update=[]
                        )
                        nc.register_instruction(nop)
                        new_insts.append(nop)
                    si.on_wait = keep
                new_insts.append(inst)
            blk.instructions[:] = new_insts


N = 100_000
E = 3_200_000
F = 64
C = 2
THETAS = np.array([[3.0, -3.0, 0.75], [0.0, 3.0, -1.5], [0.0, 0.0, 0.75]])
W = 8
R = 12544
NB = 98
PAIRS_PC = R // 2          # 6272 pairs per core
PAIRS_W = PAIRS_PC // 2    # 3136 pairs per window per core
ROWS_W = PAIRS_W * 2       # 6272 rows per window per core
NSP = 7
SPB = NB // NSP            # 14 buckets per super-phase (2 PSUM banks)
CHUNK_BLOCKS = 64
SUBG = 4
NQUEUE = 4

DT_BF16 = mybir.dt.bfloat16
DT_F32 = mybir.dt.float32
DT_I16 = mybir.dt.int16
RELU = mybir.ActivationFunctionType.Relu

LAST_EXEC_NS = None
_TRACE = os.environ.get("BWGNN_TRACE", "0") == "1"


def _preprocess(src, dst):
    src = np.asarray(src).astype(np.int64).ravel()
    dst = np.asarray(dst).astype(np.int64).ravel()

    deg = np.bincount(dst, minlength=N)
    dinv = (np.clip(deg, 1, None).astype(np.float64) ** -0.5).astype(np.float32)

    odeg = np.bincount(src, minlength=N)
    order = np.argsort(-odeg, kind="stable")
    node_core = np.empty(N, dtype=np.int64)
    node_pos = np.empty(N, dtype=np.int64)
    pair_of = np.arange(N) // 2
    half_of = np.arange(N) % 2
    node_core[order] = pair_of % W
    node_pos[order] = (pair_of // W) * 2 + half_of
    perm = np.full((W, R), -1, dtype=np.int64)
    perm[node_core, node_pos] = np.arange(N)

    e_c = node_core[dst]
    e_pos = node_pos[dst]
    e_b = e_pos // 128
    e_dstp = e_pos % 128
    e_sp = e_b // SPB
    s_c = node_core[src]
    s_pos = node_pos[src]
    s_pair = s_pos // 2
    e_par = s_pos % 2
    e_w = s_pair // PAIRS_W
    e_idx = s_c * PAIRS_W + (s_pair - e_w * PAIRS_W)

    NGRP = NSP * 2 * 2 * NB
    g_key = ((e_sp * 2 + e_w) * 2 + e_par) * NB + e_b
    flat = e_c * NGRP + g_key
    bc = np.bincount(flat, minlength=W * NGRP)
    cnt = bc.reshape(W, NGRP)
    nblk_g = np.ceil(cnt.max(axis=0) / 128.0).astype(np.int64)

    blk_grp = np.repeat(np.arange(NGRP), nblk_g)
    NBLK = len(blk_grp)
    blk_off_g = np.zeros(NGRP + 1, dtype=np.int64)
    np.cumsum(nblk_g, out=blk_off_g[1:])

    blk_b = blk_grp % NB
    blk_par = (blk_grp // NB) % 2
    blk_w = (blk_grp // (NB * 2)) % 2
    blk_sp = blk_grp // (NB * 4)

    start_flag = np.zeros(NBLK, dtype=bool)
    stop_flag = np.zeros(NBLK, dtype=bool)
    for b in range(NB):
        ids = np.nonzero(blk_b == b)[0]
        assert len(ids) > 0, f"bucket {b} has no blocks"
        start_flag[ids[0]] = True
        stop_flag[ids[-1]] = True

    chunks = []
    for sp in range(NSP):
        for w in range(2):
            m = (blk_sp == sp) & (blk_w == w)
            ids = np.nonzero(m)[0]
            if len(ids) == 0:
                continue
            assert np.all(np.diff(ids) == 1)
            lo, hi = ids[0], ids[-1] + 1
            for x in range(lo, hi, CHUNK_BLOCKS):
                chunks.append((x, min(x + CHUNK_BLOCKS, hi), w, sp))

    eo = np.argsort(flat, kind="stable")
    starts = np.zeros(W * NGRP + 1, dtype=np.int64)
    np.cumsum(bc, out=starts[1:])
    ranks = np.arange(E) - starts[flat[eo]]
    g_eo = g_key[eo]
    blk_of_e = blk_off_g[g_eo] + ranks // 128
    part_of_e = ranks % 128
    c_eo = e_c[eo]

    dstl = np.full((W, 128, NBLK), -1.0, dtype=np.float32)
    dstl[c_eo, part_of_e, blk_of_e] = e_dstp[eo].astype(np.float32)

    idxflat = np.zeros((W, NBLK * 128), dtype=np.int16)
    idxflat[c_eo, blk_of_e * 128 + part_of_e] = e_idx[eo].astype(np.int16)
    idx16 = np.zeros((W, 128, NBLK * 8), dtype=np.int16)
    for c in range(W):
        wrapped = idxflat[c].reshape(NBLK, 8, 16)
        wr = wrapped.transpose(2, 0, 1).reshape(16, NBLK * 8)
        idx16[c] = np.tile(wr, (8, 1))

    return dict(
        dinv=dinv, perm=perm,
        NBLK=NBLK, blk_b=blk_b, blk_par=blk_par, blk_w=blk_w, blk_sp=blk_sp,
        start_flag=start_flag, stop_flag=stop_flag, chunks=chunks,
        dstl=dstl, idx16=idx16,
    )


def _build(meta):
    NBLK = meta["NBLK"]
    blk_b = meta["blk_b"]
    blk_par = meta["blk_par"]
    start_flag = meta["start_flag"]
    stop_flag = meta["stop_flag"]
    chunks = meta["chunks"]

    nc = bacc.Bacc(
        None,
        target_bir_lowering=False,
        num_swdge_queues=NQUEUE,
        dynamic_dma_scratch_size=32768,
    )

    xT = nc.declare_dram_parameter("xT", [F, R], DT_BF16, isOutput=False)
    dinv_in = nc.declare_dram_parameter("dinv", [128, NB], DT_F32, isOutput=False)
    w1 = nc.declare_dram_parameter("w1", [F, F], DT_BF16, isOutput=False)
    w2 = nc.declare_dram_parameter("w2", [F, F], DT_BF16, isOutput=False)
    w3p = nc.declare_dram_parameter("w3p", [F, 3 * F], DT_BF16, isOutput=False)
    w4 = nc.declare_dram_parameter("w4", [F, C], DT_BF16, isOutput=False)
    b1 = nc.declare_dram_parameter("b1", [F, 1], DT_F32, isOutput=False)
    b2 = nc.declare_dram_parameter("b2", [F, 1], DT_F32, isOutput=False)
    b3 = nc.declare_dram_parameter("b3", [F, 1], DT_F32, isOutput=False)
    b4 = nc.declare_dram_parameter("b4", [C, 1], DT_F32, isOutput=False)
    id64 = nc.declare_dram_parameter("id64", [F, F], DT_BF16, isOutput=False)
    id128 = nc.declare_dram_parameter("id128", [128, 128], DT_BF16, isOutput=False)
    iota = nc.declare_dram_parameter("iota", [128, 128], DT_BF16, isOutput=False)
    idx_in = nc.declare_dram_parameter("idx", [128, NBLK * 8], DT_I16, isOutput=False)
    dstl_in = nc.declare_dram_parameter("dstl", [128, NBLK], DT_F32, isOutput=False)
    outT = nc.declare_dram_parameter("outT", [C, R], DT_F32, isOutput=True)
    DEBUG = os.environ.get("BWGNN_DEBUG", "0") == "1"
    if DEBUG:
        dbg_h = nc.declare_dram_parameter("dbg_h", [128, NB * F], DT_BF16, isOutput=True)
        dbg_g1 = nc.declare_dram_parameter("dbg_g1", [128, NB * F], DT_BF16, isOutput=True)
        dbg_gt = nc.declare_dram_parameter("dbg_gt", [128, CHUNK_BLOCKS * 128], DT_BF16, isOutput=True)
        dbg_oh = nc.declare_dram_parameter("dbg_oh", [128, 128], DT_BF16, isOutput=True)
        dbg_agg = nc.declare_dram_parameter("dbg_agg", [128, SPB * F], DT_F32, isOutput=True)

    cc1 = nc.dram_tensor("cc1", [R, F], DT_BF16)
    cc2 = nc.dram_tensor("cc2", [R, F], DT_BF16)
    t1 = [nc.dram_tensor(f"t1w{w}", [W * ROWS_W, F], DT_BF16, addr_space="Shared")
          for w in range(2)]
    t2 = [nc.dram_tensor(f"t2w{w}", [W * ROWS_W, F], DT_BF16, addr_space="Shared")
          for w in range(2)]
    rg = [list(range(W))]

    with tile.TileContext(nc) as tc:
        with (
            tc.tile_pool(name="const", bufs=1) as constp,
            tc.tile_pool(name="persist", bufs=1) as persist,
        ):
            def cload(nm, shape, dtype, srcap):
                t = constp.tile(shape, dtype, name=nm, tag=nm)
                nc.sync.dma_start(out=t[:], in_=srcap)
                return t

            w1s = cload("w1s", [F, F], DT_BF16, w1[:])
            w2s = cload("w2s", [F, F], DT_BF16, w2[:])
            w3s = cload("w3s", [F, 3 * F], DT_BF16, w3p[:])
            w4s = cload("w4s", [F, C], DT_BF16, w4[:])
            b1s = cload("b1s", [F, 1], DT_F32, b1[:])
            b2s = cload("b2s", [F, 1], DT_F32, b2[:])
            b3s = cload("b3s", [F, 1], DT_F32, b3[:])
            b4s = cload("b4s", [C, 1], DT_F32, b4[:])
            id64s = cload("id64s", [F, F], DT_BF16, id64[:])
            id128s = cload("id128s", [128, 128], DT_BF16, id128[:])
            iotas = cload("iotas", [128, 128], DT_BF16, iota[:])
            dinvs = cload("dinvs", [128, NB], DT_F32, dinv_in[:])
            dstls = cload("dstls", [128, NBLK], DT_F32, dstl_in[:])
            idxs = cload("idxs", [128, NBLK * 8], DT_I16, idx_in[:])

            h_node = persist.tile([128, NB * F], DT_BF16)
            g1_node = persist.tile([128, NB * F], DT_BF16)

            # ---------------- phase A ----------------
            with (
                tc.tile_pool(name="pA", bufs=3) as pA,
                tc.tile_pool(name="psA", bufs=2, space="PSUM") as psA,
            ):
                for b in range(NB):
                    xt = pA.tile([F, 128], DT_BF16, tag="xt")
                    nc.sync.dma_start(out=xt[:], in_=xT[:, b * 128:(b + 1) * 128])
                    ps1 = psA.tile([F, 128], DT_F32, tag="ps1")
                    nc.tensor.matmul(ps1[:], w1s[:], xt[:], start=True, stop=True)
                    h1 = pA.tile([F, 128], DT_BF16, tag="h1")
                    nc.scalar.activation(h1[:], ps1[:], RELU, bias=b1s[:])
                    ps2 = psA.tile([F, 128], DT_F32, tag="ps2")
                    nc.tensor.matmul(ps2[:], w2s[:], h1[:], start=True, stop=True)
                    h2 = pA.tile([F, 128], DT_BF16, tag="h2")
                    nc.scalar.activation(h2[:], ps2[:], RELU, bias=b2s[:])
                    psT = psA.tile([128, F], DT_BF16, tag="psT")
                    nc.tensor.transpose(psT[:], h2[:], id64s[:])
                    nc.vector.tensor_copy(
                        out=h_node[:, b * F:(b + 1) * F], in_=psT[:]
                    )
                    tw = pA.tile([128, F], DT_BF16, tag="tw")
                    nc.vector.tensor_scalar(
                        out=tw[:], in0=h_node[:, b * F:(b + 1) * F],
                        scalar1=dinvs[:, b:b + 1], scalar2=None,
                        op0=mybir.AluOpType.mult,
                    )
                    nc.scalar.dma_start(
                        out=cc1[b * 128:(b + 1) * 128, :], in_=tw[:]
                    )
                    if b == 48:
                        nc.gpsimd.collective_compute(
                            "AllGather", mybir.AluOpType.bypass,
                            replica_groups=rg,
                            ins=[cc1[0:ROWS_W, :]], outs=[t1[0][:]],
                        )
                nc.gpsimd.collective_compute(
                    "AllGather", mybir.AluOpType.bypass, replica_groups=rg,
                    ins=[cc1[ROWS_W:R, :]], outs=[t1[1][:]],
                )

            # ---------------- hops ----------------
            with (
                tc.tile_pool(name="gt", bufs=4) as gtp,
                tc.tile_pool(name="oh", bufs=36) as ohp,
                tc.tile_pool(name="epi", bufs=2) as epi,
                tc.tile_pool(name="psAgg", bufs=2, space="PSUM") as psAgg,
                tc.tile_pool(name="psC", bufs=1, space="PSUM") as psC,
            ):
                def emit_epilogue(sp, agg, cc_out, tabs_next, is_last):
                    scaled = epi.tile([128, SPB * F], DT_BF16, tag="scaled")
                    for bl in range(SPB):
                        b = sp * SPB + bl
                        nc.vector.tensor_scalar(
                            out=scaled[:, bl * F:(bl + 1) * F],
                            in0=agg[:, bl * F:(bl + 1) * F],
                            scalar1=dinvs[:, b:b + 1], scalar2=None,
                            op0=mybir.AluOpType.mult,
                        )
                    cols = slice(sp * SPB * F, (sp + 1) * SPB * F)
                    if not is_last:
                        nc.vector.tensor_tensor(
                            out=g1_node[:, cols],
                            in0=h_node[:, cols], in1=scaled[:],
                            op=mybir.AluOpType.subtract,
                        )
                        tw2 = epi.tile([128, SPB * F], DT_BF16, tag="tw2")
                        for bl in range(SPB):
                            b = sp * SPB + bl
                            nc.vector.tensor_scalar(
                                out=tw2[:, bl * F:(bl + 1) * F],
                                in0=g1_node[:, b * F:(b + 1) * F],
                                scalar1=dinvs[:, b:b + 1], scalar2=None,
                                op0=mybir.AluOpType.mult,
                            )
                        r0 = sp * SPB * 128
                        nc.scalar.dma_start(
                            out=cc_out[
                                r0:r0 + SPB * 128, :
                            ].rearrange("(b p) f -> p b f", p=128),
                            in_=tw2[:].rearrange("p (b f) -> p b f", f=F),
                        )
                        # fire the window AllGather once its rows are
                        # complete (window w ends at bucket 49w+48)
                        for wo in range(2):
                            last_b = 49 * wo + 48
                            if sp * SPB <= last_b < (sp + 1) * SPB:
                                nc.gpsimd.collective_compute(
                                    "AllGather", mybir.AluOpType.bypass,
                                    replica_groups=rg,
                                    ins=[cc_out[wo * ROWS_W:(wo + 1) * ROWS_W, :]],
                                    outs=[tabs_next[wo][:]],
                                )
                    else:
                        g2sp = epi.tile([128, SPB * F], DT_BF16, tag="g2sp")
                        nc.vector.tensor_tensor(
                            out=g2sp[:],
                            in0=g1_node[:, cols], in1=scaled[:],
                            op=mybir.AluOpType.subtract,
                        )
                        for bl in range(SPB):
                            b = sp * SPB + bl
                            psg = psC.tile([F, 128], DT_BF16, tag="psg")
                            nc.tensor.transpose(
                                psg[:], h_node[:, b * F:(b + 1) * F],
                                id128s[:],
                            )
                            hTt = epi.tile([F, 128], DT_BF16, tag="hTt")
                            nc.scalar.activation(
                                hTt[:], psg[:],
                                mybir.ActivationFunctionType.Copy,
                            )
                            psg2 = psC.tile([F, 128], DT_BF16, tag="psg2")
                            nc.tensor.transpose(
                                psg2[:], g1_node[:, b * F:(b + 1) * F],
                                id128s[:],
                            )
                            g1Tt = epi.tile([F, 128], DT_BF16, tag="g1Tt")
                            nc.scalar.activation(
                                g1Tt[:], psg2[:],
                                mybir.ActivationFunctionType.Copy,
                            )
                            psg3 = psC.tile([F, 128], DT_BF16, tag="psg")
                            nc.tensor.transpose(
                                psg3[:], g2sp[:, bl * F:(bl + 1) * F],
                                id128s[:],
                            )
                            g2Tt = epi.tile([F, 128], DT_BF16, tag="g2Tt")
                            nc.vector.tensor_copy(out=g2Tt[:], in_=psg3[:])
                            psZ = psC.tile([F, 128], DT_F32, tag="psZ")
                            nc.tensor.matmul(
                                psZ[:], w3s[:, 0:F], hTt[:],
                                start=True, stop=False,
                            )
                            nc.tensor.matmul(
                                psZ[:], w3s[:, F:2 * F], g1Tt[:],
                                start=False, stop=False,
                            )
                            nc.tensor.matmul(
                                psZ[:], w3s[:, 2 * F:3 * F], g2Tt[:],
                                start=False, stop=True,
                            )
                            zb = epi.tile([F, 128], DT_BF16, tag="zb")
                            nc.scalar.activation(zb[:], psZ[:], RELU, bias=b3s[:])
                            psO = psC.tile([C, 128], DT_F32, tag="psO")
                            nc.tensor.matmul(
                                psO[:], w4s[:], zb[:], start=True, stop=True
                            )
                            ob = epi.tile([C, 128], DT_F32, tag="ob")
                            nc.vector.tensor_scalar(
                                out=ob[:], in0=psO[:],
                                scalar1=b4s[:], scalar2=None,
                                op0=mybir.AluOpType.add,
                            )
                            nc.scalar.dma_start(
                                out=outT[:, b * 128:(b + 1) * 128],
                                in_=ob[:],
                            )

                def run_hop(tabs, cc_out, tabs_next, is_last):
                    qctr = 0
                    agg = None
                    pending = None        # (sp, agg) awaiting epilogue
                    defer = 0
                    for ci, (lo, hi, w, sp) in enumerate(chunks):
                        if agg is None:
                            agg = psAgg.tile([128, SPB * F], DT_F32, tag="agg")
                            nc.vector.memset(agg[:], 0.0)
                        nblk = hi - lo
                        n = nblk * 128
                        gt = gtp.tile(
                            [128, CHUNK_BLOCKS * 128], DT_BF16, tag="gt"
                        )
                        step = (nblk + SUBG - 1) // SUBG
                        for s in range(SUBG):
                            s_lo = lo + s * step
                            s_hi = min(hi, s_lo + step)
                            if s_hi <= s_lo:
                                break
                            ns = (s_hi - s_lo) * 128
                            nc.gpsimd.dma_gather(
                                gt[:, (s_lo - lo) * 128:(s_hi - lo) * 128]
                                .rearrange("p (c f) -> p c f", f=128),
                                tabs[w][:].rearrange(
                                    "(a two) f -> a (two f)", two=2
                                ),
                                idxs[:, s_lo * 8: s_hi * 8],
                                num_idxs=ns,
                                num_idxs_reg=ns,
                                elem_size=128,
                                single_packet=False,
                                queue_num=s,
                            )
                        qctr += 1
                        for blk in range(lo, hi):
                            i = blk - lo
                            b = int(blk_b[blk])
                            par = int(blk_par[blk])
                            oh = ohp.tile([128, 128], DT_BF16, tag="oh")
                            nc.vector.tensor_scalar(
                                out=oh[:], in0=iotas[:],
                                scalar1=dstls[:, blk:blk + 1], scalar2=None,
                                op0=mybir.AluOpType.is_equal,
                            )
                            bl = b - sp * SPB
                            nc.tensor.matmul(
                                agg[:, bl * F:(bl + 1) * F],
                                oh[:],
                                gt[:, i * 128 + 64 * par: i * 128 + 64 * par + 64],
                                start=False,
                                stop=bool(stop_flag[blk]),
                                skip_group_check=True,
                            )
                        if pending is not None:
                            defer -= 1
                            if defer <= 0:
                                emit_epilogue(pending[0], pending[1],
                                              cc_out, tabs_next, is_last)
                                pending = None
                        last_of_sp = (ci + 1 == len(chunks)) or (
                            chunks[ci + 1][3] != sp
                        )
                        if last_of_sp:
                            assert pending is None
                            pending = (sp, agg)
                            defer = 2
                            agg = None
                    if pending is not None:
                        emit_epilogue(pending[0], pending[1],
                                      cc_out, tabs_next, is_last)

                run_hop(t1, cc2, t2, is_last=False)
                run_hop(t2, None, None, is_last=True)
                if DEBUG:
                    nc.sync.dma_start(out=dbg_h[:], in_=h_node[:])
                    nc.sync.dma_start(out=dbg_g1[:], in_=g1_node[:])

    nc.compile()
    split_waits(nc)
    return nc


def kernel(in_feat, src, dst, W1, b1, W2, b2, W3, b3, W4, b4):
    global LAST_EXEC_NS
    in_feat = np.asarray(in_feat, dtype=np.float32)
    meta = _preprocess(src, dst)
    nc = _build(meta)

    dinv, perm = meta["dinv"], meta["perm"]
    W1 = np.asarray(W1, np.float32)
    W2 = np.asarray(W2, np.float32)
    W3 = np.asarray(W3, np.float32)
    W4 = np.asarray(W4, np.float32)
    b1v = np.asarray(b1, np.float32).reshape(F, 1)
    b2v = np.asarray(b2, np.float32).reshape(F, 1)
    b3v = np.asarray(b3, np.float32).reshape(F, 1)
    b4v = np.asarray(b4, np.float32).reshape(C, 1)
    w3p = np.zeros((F, 3 * F), np.float32)
    for j in range(3):
        acc = np.zeros((F, F), np.float32)
        for i in range(3):
            acc += THETAS[i, j] * W3[i * F:(i + 1) * F, :]
        w3p[:, j * F:(j + 1) * F] = acc

    id64 = np.eye(F, dtype=np.float32).astype(ml_dtypes.bfloat16)
    id128 = np.eye(128, dtype=np.float32).astype(ml_dtypes.bfloat16)
    iota = np.tile(np.arange(128, dtype=np.float32), (128, 1)).astype(
        ml_dtypes.bfloat16
    )

    in_maps = []
    for c in range(W):
        pm = perm[c]
        real = pm >= 0
        xc = np.zeros((R, F), np.float32)
        xc[real] = in_feat[pm[real]]
        dv = np.zeros(R, np.float32)
        dv[real] = dinv[pm[real]]
        in_maps.append(
            {
                "xT": np.ascontiguousarray(xc.T).astype(ml_dtypes.bfloat16),
                "dinv": np.ascontiguousarray(
                    dv.reshape(NB, 128).T
                ).astype(np.float32),
                "w1": W1.astype(ml_dtypes.bfloat16),
                "w2": W2.astype(ml_dtypes.bfloat16),
                "w3p": w3p.astype(ml_dtypes.bfloat16),
                "w4": W4.astype(ml_dtypes.bfloat16),
                "b1": b1v, "b2": b2v, "b3": b3v, "b4": b4v,
                "id64": id64, "id128": id128, "iota": iota,
                "idx": meta["idx16"][c],
                "dstl": np.ascontiguousarray(meta["dstl"][c]),
            }
        )

    res = run_bass_kernel_spmd(nc, in_maps, core_ids=list(range(W)), trace=_TRACE)
    LAST_EXEC_NS = res.exec_time_ns

    out = np.empty((N, C), dtype=np.float32)
    for c in range(W):
        oT = res.results[c]["outT"]          # [C, R]
        pm = perm[c]
        real = pm >= 0
        out[pm[real]] = oT.T[real]
    return out
